# revision 2
# baseline (speedup 1.0000x reference)
"""Decoder kernel for trn2 — optimized phase D (v2).

Sharding: data-parallel over batch N=64 -> 8 sequences per core.
Per core:
  B. attention (energy/exp-mask/context) per sequence  [f32r matmuls]
  C. z1x = [ce|ctx/s] @ W_ih1p.T (+b1) batched, n-major rows -> DRAM (bf16)
  D. LSTM chains, decoupled pipelines:
     - layout: z/gates/c/h tiles are [128, 128]-style with batch rows at
       partition 32j+r (j = feature chunk, r = batch 0..8); full-partition
       ACT/DVE ops (garbage rows processed for free)
     - one PE transpose per chain per window gives feature-major h for the
       next window's stationaries (no padded stationary copies)
     - chain2 lags chain1 by one window; h2 written straight into h2T_all
  E. vocab projection (bf16), (t,n)-major rows, streamed w_outT in
     1000-col slabs
"""
import sys
sys.path.insert(0, '/opt/trn_rl_repo')
import numpy as np
import ml_dtypes
from concourse import bass, bacc, mybir
from concourse.tile import TileContext
from concourse import masks

F32, F32R, BF16 = mybir.dt.float32, mybir.dt.float32r, mybir.dt.bfloat16
AF = mybir.ActivationFunctionType
ALU = mybir.AluOpType

N_CORES = 8
T, K, V, H, L, VOCAB = 512, 512, 512, 512, 256, 10000
NL = 8
R = NL * L       # 2048 rows per core
MASK_NEG = -30.0
VB = 500         # vocab slice (20 slices, loaded in 1000-col pairs)


PERM = np.concatenate([
    np.concatenate([gate * 512 + np.arange(128 * j, 128 * (j + 1))
                    for gate in (0, 1, 3, 2)])       # i, f, o, g
    for j in range(4)])


def _bf(x):
    return np.ascontiguousarray(x).astype(ml_dtypes.bfloat16)


def host_prep(inputs):
    key = np.asarray(inputs["key"], np.float32)
    values = np.asarray(inputs["values"], np.float32)
    text = np.asarray(inputs["text"])
    text_lens = np.asarray(inputs["text_lens"])
    emb = np.asarray(inputs["emb"], np.float32)
    w_ih1 = np.asarray(inputs["w_ih1"], np.float32)
    w_hh1 = np.asarray(inputs["w_hh1"], np.float32)
    w_ih2 = np.asarray(inputs["w_ih2"], np.float32)
    w_hh2 = np.asarray(inputs["w_hh2"], np.float32)
    b1 = (np.asarray(inputs["b_ih1"], np.float32)
          + np.asarray(inputs["b_hh1"], np.float32))
    b2 = (np.asarray(inputs["b_ih2"], np.float32)
          + np.asarray(inputs["b_hh2"], np.float32))
    w_out = np.asarray(inputs["w_out"], np.float32)

    w_ih1p, w_hh1p, b1p = w_ih1[PERM], w_hh1[PERM], b1[PERM]
    w_ih2p, w_hh2p, b2p = w_ih2[PERM], w_hh2[PERM], b2[PERM]

    mask = (np.arange(T)[None, :] < text_lens[:, None])
    maskb = np.where(mask, 0.0, MASK_NEG).astype(np.float32)

    ce_all = emb[text[:, :L]]

    shared = {
        "w1ceT": np.ascontiguousarray(w_ih1p[:, :512].T),
        "w1ctxT": np.ascontiguousarray(w_ih1p[:, 512:].T),
        "b1": b1p.reshape(1, 2048),
        "whh1T": _bf(w_hh1p.T),          # (512, 2048) bf16
        "wih2T": _bf(w_ih2p.T),
        "whh2T": _bf(w_hh2p.T),
        "b2bf": _bf(b2p.reshape(1, 2048)),
        "w_outT": _bf(w_out.T),          # (1024, 10000) bf16
    }
    in_maps = []
    for c in range(N_CORES):
        sl = slice(8 * c, 8 * c + 8)
        ceT = np.ascontiguousarray(ce_all[sl].reshape(R, H).T)
        keyT = np.ascontiguousarray(key[:, sl, :].transpose(1, 2, 0))
        vals = np.ascontiguousarray(values[:, sl, :])
        valT = _bf(values[:L, sl, :].reshape(R, V).T)
        m = {"ceT": ceT, "keyT": keyT, "vals": vals, "valT": valT,
             "maskb": np.ascontiguousarray(maskb[sl]), **shared}
        in_maps.append(m)
    return in_maps


def build(debug_outputs=(), upto="E", with_b1=True, with_b2=False, reps=1):
    nc = bacc.Bacc("TRN2", target_bir_lowering=False, debug=False,
                   num_devices=N_CORES)
    d = {}
    def din(name, shape, dt=F32):
        d[name] = nc.dram_tensor(name, list(shape), dt, kind="ExternalInput")
    din("ceT", (H, R)); din("keyT", (NL, K, T)); din("vals", (T, NL, V))
    din("maskb", (NL, T))  # L-indep
    din("w1ceT", (512, 2048)); din("w1ctxT", (512, 2048))
    din("b1", (1, 2048))
    din("whh1T", (512, 2048), BF16); din("wih2T", (512, 2048), BF16)
    din("whh2T", (512, 2048), BF16); din("b2bf", (1, 2048), BF16)
    din("valT", (V, R), BF16); din("w_outT", (1024, VOCAB), BF16)

    out = nc.dram_tensor("out", [NL, L, VOCAB], F32, kind="ExternalOutput")
    dbg = {}
    shapes = {"att": (NL, T, L), "ctxT": (V, R), "recip": (NL, L),
              "z1x": (L, NL, 2048), "hh": (L, 2 * NL, H),
              "gg1": (L, NL, 2048), "zq0": (NL, 2 * 2048),
              "ht0": (128, 32)}
    for name in debug_outputs:
        dbg[name] = nc.dram_tensor("dbg_" + name, list(shapes[name]), F32,
                                   kind="ExternalOutput")

    with TileContext(nc) as tc:
        for _ in range(reps):
            build_body(nc, tc, d, out, dbg, upto, with_b1, with_b2)
    return nc


def build_body(nc, tc, d, out, dbg, upto, with_b1, with_b2):
    from contextlib import ExitStack
    ctx = ExitStack()
    pool = ctx.enter_context(tc.tile_pool(name="main", bufs=1))
    dramp = ctx.enter_context(tc.tile_pool(name="drp", bufs=1, space="DRAM"))

    # ---- constants ----
    ones_f = pool.tile([128, 128], F32, tag="ones_f")
    nc.gpsimd.memset(ones_f[:], 1.0)
    ones_row = pool.tile([1, 128], F32R, tag="ones_row")
    nc.vector.tensor_copy(ones_row[:], ones_f[0:1, :])
    ones_col = pool.tile([128, 1], F32R, tag="ones_col")
    nc.vector.tensor_copy(ones_col[:], ones_f[:, 0:1])
    ident8f = pool.tile([8, 8], F32, tag="ident8f")
    masks.make_identity(nc, ident8f[:])
    ident8b = pool.tile([8, 8], BF16, tag="ident8b")
    masks.make_identity(nc, ident8b[:])
    ident128f = pool.tile([128, 128], F32, tag="ident128f")
    masks.make_identity(nc, ident128f[:])
    onesb_row = pool.tile([1, 8], BF16, tag="onesb_row")
    nc.gpsimd.memset(onesb_row[:], 1.0)

    z1x_dram = dramp.tile([L, NL, 2048], BF16, tag="z1x_dram")

    # ================= Phase B/C scope =================
    from contextlib import ExitStack as _ES
    with tc.tile_pool(name="bc", bufs=1) as bcp, \
         tc.tile_pool(name="bcps", bufs=4, space="PSUM") as psum, \
         _ES() as bstack:
        attp = bstack.enter_context(tc.tile_pool(name="attp", bufs=2))
        ceT_sb = [bcp.tile([128, R], F32R, tag=f"ceT{kc}", name=f"ceT{kc}")
                  for kc in range(4)]
        for kc in range(4):
            nc.sync.dma_start(ceT_sb[kc][:],
                              d["ceT"][128*kc:128*(kc+1), :].bitcast(F32R))
        ctxT_sb = [bcp.tile([128, R], F32R, tag=f"ctxT{vc}", name=f"ctxT{vc}")
                   for vc in range(4)]
        recipT_sb = bcp.tile([128, 16], F32, tag="recipT")
        sums_sb = bcp.tile([1, NL * L], F32, tag="sums")

        # ---- Phase B: attention ----
        for n in range(NL):
            keyT_n = attp.tile([128, 4 * T], F32R, tag="keyT_n")
            for kc in range(4):
                nc.sync.dma_start(keyT_n[:, T*kc:T*(kc+1)],
                                  d["keyT"][n, 128*kc:128*(kc+1), :].bitcast(F32R))
            maskb_n = attp.tile([128, 4], F32, tag="maskb_n")
            nc.sync.dma_start(maskb_n[:],
                              d["maskb"][n, :].rearrange("(a b) -> b a", b=128))
            att_n = attp.tile([128, 4 * L], F32R, tag="att_n")
            for tch in range(4):
                ep = psum.tile([128, L], F32, tag="mm")
                for kc in range(4):
                    nc.tensor.matmul(ep[:],
                                     keyT_n[:, T*kc+128*tch:T*kc+128*tch+128],
                                     ceT_sb[kc][:, L*n:L*(n+1)],
                                     start=(kc == 0), stop=(kc == 3))
                nc.scalar.activation(att_n[:, L*tch:L*(tch+1)], ep[:], AF.Exp,
                                     bias=maskb_n[:, tch:tch+1])
            sp = psum.tile([1, L], F32, tag="sp", bufs=1)
            for tch in range(4):
                nc.tensor.matmul(sp[:], ones_col[:, :1],
                                 att_n[:, L*tch:L*(tch+1)],
                                 start=(tch == 0), stop=(tch == 3))
            nc.scalar.activation(sums_sb[0:1, L*n:L*(n+1)], sp[:], AF.Copy)
            vals_n = attp.tile([128, 4 * V], F32R, tag="vals_n")
            for tch in range(4):
                nc.gpsimd.dma_start(vals_n[:, V*tch:V*(tch+1)],
                                    d["vals"][128*tch:128*(tch+1), n, :].bitcast(F32R))
            for vc in range(4):
                cp = psum.tile([128, L], F32, tag="mm")
                for tch in range(4):
                    nc.tensor.matmul(cp[:],
                                     vals_n[:, V*tch+128*vc:V*tch+128*vc+128],
                                     att_n[:, L*tch:L*(tch+1)],
                                     start=(tch == 0), stop=(tch == 3))
                nc.scalar.activation(ctxT_sb[vc][:, L*n:L*(n+1)], cp[:], AF.Copy)
            if "att" in dbg:
                af = attp.tile([128, 4 * L], F32, tag="dbgf", bufs=1)
                nc.vector.tensor_copy(af[:], att_n[:].bitcast(F32))
                for tch in range(4):
                    nc.sync.dma_start(dbg["att"][n, 128*tch:128*(tch+1), :],
                                      af[:, L*tch:L*(tch+1)])

        recip_nb = bcp.tile([1, NL * L], F32, tag="recip_nb")
        nc.vector.reciprocal(recip_nb[:], sums_sb[:])
        rcols = min(L, 128)
        nchk = max(L // 128, 1)
        for n in range(NL):
            for hh in range(nchk):
                rp = psum.tile([128, 1], F32, tag="rp", bufs=2)
                nc.tensor.matmul(rp[:rcols, :],
                                 recip_nb[0:1, L*n+rcols*hh:L*n+rcols*(hh+1)],
                                 ident8f[0:1, 0:1], is_transpose=True)
                nc.scalar.activation(recipT_sb[:rcols, 8*hh+n:8*hh+n+1],
                                     rp[:rcols, :], AF.Copy)
        if "recip" in dbg:
            nc.sync.dma_start(dbg["recip"][:, :], recip_nb[:])
        if "ctxT" in dbg:
            for vc in range(4):
                cf = attp.tile([128, R], F32, tag="dbgf", bufs=1)
                nc.vector.tensor_copy(cf[:], ctxT_sb[vc][:].bitcast(F32))
                nc.sync.dma_start(dbg["ctxT"][128*vc:128*(vc+1), :], cf[:])
        if upto == "B":
            ctx.close(); return

        # ---- Phase C: z1x (n-major rows; -> z1x_dram step-major bf16) ----
        bstack.close()   # free attention pools before the z1x weights
        zstack = _ES()
        zwp = zstack.enter_context(tc.tile_pool(name="zwp", bufs=1))
        zxp = zstack.enter_context(tc.tile_pool(name="zxp", bufs=4))
        w1ce_sb = [zwp.tile([128, 2048], F32R, tag=f"w1ce{kc}", name=f"w1ce{kc}")
                   for kc in range(4)]
        w1ctx_sb = [zwp.tile([128, 2048], F32R, tag=f"w1ctx{kc}", name=f"w1ctx{kc}")
                    for kc in range(4)]
        b1_sb = zwp.tile([1, 2048], F32R, tag="b1_sb")
        nc.sync.dma_start(b1_sb[:], d["b1"][:].bitcast(F32R))
        for kc in range(4):
            nc.sync.dma_start(w1ce_sb[kc][:],
                              d["w1ceT"][128*kc:128*(kc+1), :].bitcast(F32R))
            nc.sync.dma_start(w1ctx_sb[kc][:],
                              d["w1ctxT"][128*kc:128*(kc+1), :].bitcast(F32R))
        lchunks = max(L // 128, 1)
        crows = min(L, 128)  # rows per (n, lchunk) piece
        for rc in range(R // crows):
            n_of = rc // lchunks
            lh = rc % lchunks
            rs = slice(crows * rc, crows * (rc + 1))
            recip_col = recipT_sb[:crows, 8*lh + n_of: 8*lh + n_of + 1]
            zx_bf = zxp.tile([128, 2048], BF16, tag="zx_bf", bufs=3)
            for j in range(4):
                fs = slice(512 * j, 512 * (j + 1))
                pce = psum.tile([128, 512], F32, tag="mm")
                first = True
                if with_b1:
                    nc.tensor.matmul(pce[:crows, :], ones_row[:1, :crows], b1_sb[:1, fs],
                                     start=True, stop=False)
                    first = False
                for kc in range(4):
                    nc.tensor.matmul(pce[:crows, :], ceT_sb[kc][:, rs],
                                     w1ce_sb[kc][:, fs],
                                     start=(first and kc == 0), stop=(kc == 3))
                pctx = psum.tile([128, 512], F32, tag="mm")
                for kc in range(4):
                    nc.tensor.matmul(pctx[:crows, :], ctxT_sb[kc][:, rs],
                                     w1ctx_sb[kc][:, fs],
                                     start=(kc == 0), stop=(kc == 3))
                zce_sb = zxp.tile([128, 512], F32, tag="zce_sb")
                nc.scalar.activation(zce_sb[:crows, :], pce[:crows, :], AF.Copy)
                nc.vector.scalar_tensor_tensor(out=zx_bf[:crows, fs], in0=pctx[:crows, :],
                                               scalar=recip_col, in1=zce_sb[:crows, :],
                                               op0=ALU.mult, op1=ALU.add)
            # rows (n_of, l=crows*lh + p) -> z1x_dram[l, n, f]
            nc.sync.dma_start(z1x_dram[crows*lh:crows*(lh+1), n_of, :],
                              zx_bf[:crows, :])
        if "z1x" in dbg:
            for t in range(0, L, 8):
                zrb = zxp.tile([8 * NL, 2048], BF16, tag="zrb", bufs=1)
                nc.sync.dma_start(zrb[:], z1x_dram[t:t+8, :, :])
                zrf = zxp.tile([8 * NL, 2048], F32, tag="zrf", bufs=1)
                nc.vector.tensor_copy(zrf[:], zrb[:])
                nc.sync.dma_start(dbg["z1x"][t:t+8, :, :], zrf[:])
        zstack.close()
    if upto == "C":
        ctx.close(); return

    # ================= Phase D: decoupled LSTM chains =================
    # h2T_all[p, (t, k, n)]: p = feature within chunk, k = feature chunk,
    # t = step, n = batch row. Written directly by chain2's transpose cast.
    h2T_all = pool.tile([128, 4 * R], BF16, tag="h2T_all")
    h2T_v = h2T_all[:].rearrange("p (k r) -> p k r", k=4)

    with tc.tile_pool(name="dph", bufs=1) as dph, \
         tc.tile_pool(name="dst", bufs=2) as dst, \
         tc.tile_pool(name="zqp", bufs=3) as zqp, \
         tc.tile_pool(name="zps", bufs=1, space="PSUM") as zps, \
         tc.tile_pool(name="tps", bufs=1, space="PSUM") as tps, \
         tc.tile_pool(name="wvp", bufs=16) as wvp, \
         tc.tile_pool(name="osp", bufs=6) as osp, \
         tc.tile_pool(name="eps", bufs=4, space="PSUM") as eps:
        whh1_sb = [dph.tile([128, 2048], BF16, tag=f"whh1_{k}", name=f"whh1_{k}")
                   for k in range(4)]
        wih2_sb = [dph.tile([128, 2048], BF16, tag=f"wih2_{k}", name=f"wih2_{k}")
                   for k in range(4)]
        whh2_sb = [dph.tile([128, 2048], BF16, tag=f"whh2_{k}", name=f"whh2_{k}")
                   for k in range(4)]
        for k in range(4):
            nc.sync.dma_start(whh1_sb[k][:], d["whh1T"][128*k:128*(k+1), :])
            nc.sync.dma_start(wih2_sb[k][:], d["wih2T"][128*k:128*(k+1), :])
            nc.sync.dma_start(whh2_sb[k][:], d["whh2T"][128*k:128*(k+1), :])
        b2_sb = dph.tile([1, 2048], BF16, tag="b2_sb")
        nc.sync.dma_start(b2_sb[:], d["b2bf"][:])

        valT_sb = dph.tile([128, 4 * R], BF16, tag="valT_sb")
        valT_v = valT_sb[:].rearrange("p (k r) -> p k r", k=4)
        for k in range(4):
            nc.sync.dma_start(valT_v[:, k, :], d["valT"][128*k:128*(k+1), :])

        def emit_vocab(rc_lo, rc_hi, tagp):
            for vp in range(VOCAB // (2 * VB)):
                wts = [wvp.tile([128, 2 * VB], BF16, tag="wv",
                                name=f"wv{tagp}_{vp}_{k}") for k in range(8)]
                for k in range(8):
                    nc.gpsimd.dma_start(wts[k][:],
                                        d["w_outT"][128*k:128*(k+1),
                                                    2*VB*vp:2*VB*(vp+1)])
                for rc in range(rc_lo, rc_hi):
                    osb = osp.tile([128, 2 * VB], F32, tag="osb")
                    for hv in range(2):
                        pv = eps.tile([128, VB], F32, tag="pv")
                        for k in range(4):
                            nc.tensor.matmul(pv[:],
                                             h2T_v[:, k, 128*rc:128*(rc+1)],
                                             wts[k][:, VB*hv:VB*(hv+1)],
                                             start=(k == 0), stop=False)
                        for k in range(4):
                            nc.tensor.matmul(pv[:],
                                             valT_v[:, k, 128*rc:128*(rc+1)],
                                             wts[4+k][:, VB*hv:VB*(hv+1)],
                                             start=False, stop=(k == 3))
                        if (rc + hv) % 2 == 0:
                            nc.scalar.activation(osb[:, VB*hv:VB*(hv+1)],
                                                 pv[:], AF.Copy)
                        else:
                            nc.vector.tensor_copy(osb[:, VB*hv:VB*(hv+1)],
                                                  pv[:])
                    nc.sync.dma_start(
                        out[0:NL, 16*rc:16*(rc+1), 2*VB*vp:2*VB*(vp+1)]
                        .transpose([1, 0, 2]), osb[:])

        zeros32 = dph.tile([128, 32], BF16, tag="zeros32")
        nc.gpsimd.memset(zeros32[:], 0.0)
        zeros32_v = zeros32[:].rearrange("p (k c) -> p k c", k=4)
        hh1_init = dph.tile([128, 128], F32, tag="hh1_init")
        nc.vector.memset(hh1_init[:], 0.0)
        cc1_prev = dph.tile([128, 128], F32, tag="cc1_init")
        nc.vector.memset(cc1_prev[:], 0.0)
        cc2_prev = dph.tile([128, 128], F32, tag="cc2_init")
        nc.vector.memset(cc2_prev[:], 0.0)

        hh1_prev, hh2_prev = hh1_init, None
        h1T_v = None
        z1q = None

        def lstm_tail(zp, cc_prev, tag):
            """gates -> c_new, h (bf16 [128,128]); returns (cc_new, hh)."""
            gg = dst.tile([128, 512], F32, tag=f"gg{tag}")
            nc.scalar.activation(gg[:, 0:384], zp[:, 0:384], AF.Sigmoid)
            nc.scalar.activation(gg[:, 384:512], zp[:, 384:512], AF.Tanh)
            t1 = dst.tile([128, 128], F32, tag=f"t1{tag}")
            nc.vector.tensor_tensor(out=t1[:], in0=gg[:, 128:256],
                                    in1=cc_prev[:], op=ALU.mult)
            t2 = dst.tile([128, 128], F32, tag=f"t2{tag}")
            nc.vector.tensor_tensor(out=t2[:], in0=gg[:, 0:128],
                                    in1=gg[:, 384:512], op=ALU.mult)
            cc_new = dst.tile([128, 128], F32, tag=f"cc{tag}")
            nc.vector.tensor_tensor(out=cc_new[:], in0=t1[:], in1=t2[:],
                                    op=ALU.add)
            th = dst.tile([128, 128], F32, tag=f"th{tag}")
            nc.scalar.activation(th[:], cc_new[:], AF.Tanh)
            hh = dst.tile([128, 128], F32, tag=f"hh{tag}")
            nc.vector.tensor_tensor(out=hh[:], in0=gg[:, 256:384],
                                    in1=th[:], op=ALU.mult)
            return cc_new, hh, gg

        h1T_v_cur, h1T_v_prev = None, None
        for w in range(L + 2):
            c1 = w < L
            c2 = 2 <= w <= L + 1      # chain2(w) computes h2(w-2)
            # --- T_A(w): h1(w-1) -> feature-major bf16 [128,(k,8)] ---
            if w <= L:
                TA = tps.tile([128, 128], F32, tag="TA")
                nc.tensor.matmul(TA[:], hh1_prev[:], ident128f[:],
                                 is_transpose=True)
                h1T = dst.tile([128, 32], BF16, tag="h1T")
                h1T_v_prev, h1T_v_cur = (
                    h1T_v_cur, h1T[:].rearrange("p (k c) -> p k c", k=4))
                h1T_v = h1T_v_cur
                for kk in range(4):
                    nc.vector.tensor_copy(h1T[:, 8*kk:8*(kk+1)],
                                          TA[:, 32*kk:32*kk+8])

            # --- chain1: z1(w) = z1x(w) + h1(w-1) @ whh1 ---
            if c1:
                z1q = zqp.tile([8, 2048], BF16, tag="z1q")
                nc.sync.dma_start(z1q[:], z1x_dram[w, :, :])
                zp1 = zps.tile([128, 512], F32, tag="zp1")
                qof = 0
                for j in range(4):
                    nc.tensor.matmul(zp1[32*j:32*j+8, :], ident8b[:],
                                     z1q[:, qof+512*j:qof+512*(j+1)],
                                     start=True, stop=False,
                                     tile_position=(0, 32*j))
                for k in range(4):
                    for j in range(4):
                        nc.tensor.matmul(zp1[32*j:32*j+8, :],
                                         h1T_v[:, k, :],
                                         whh1_sb[k][:, 512*j:512*(j+1)],
                                         start=False, stop=(k == 3),
                                         tile_position=(0, 32*j))
                if w == 0 and "zq0" in dbg:
                    nc.gpsimd.dma_start(dbg["zq0"][:, 0:2048], z1q[:, :])
                if w == 0 and "ht0" in dbg:
                    nc.gpsimd.dma_start(dbg["ht0"][:, :], h1T[:, :])
                cc1_prev, hh1, gg1t = lstm_tail(zp1, cc1_prev, "1")
                if "gg1" in dbg:
                    for j in range(4):
                        nc.sync.dma_start(
                            dbg["gg1"][w, 0:8, 512*j:512*(j+1)],
                            gg1t[32*j:32*j+8, :])
                if "hh" in dbg:
                    for j in range(4):
                        nc.gpsimd.dma_start(dbg["hh"][w, 0:8, 128*j:128*(j+1)],
                                            hh1[32*j:32*j+8, :])
                hh1_prev = hh1

            # --- T_B(w): hh2(w-1) holds h2(w-3) -> h2T slot w-3 (w >= 3) ---
            if w >= 3:
                TB = tps.tile([128, 128], F32, tag="TB")
                nc.tensor.matmul(TB[:], hh2_prev[:], ident128f[:],
                                 is_transpose=True)
                for kk in range(4):
                    nc.vector.tensor_copy(
                        h2T_v[:, kk, 8*(w-3):8*(w-3)+8],
                        TB[:, 32*kk:32*kk+8])

            # --- chain2(w): z2(w-2) = h1(w-2) @ wih2 + h2(w-3) @ whh2 ---
            if c2:
                zp2 = zps.tile([128, 512], F32, tag="zp2")
                for k in range(4):
                    for j in range(4):
                        nc.tensor.matmul(zp2[32*j:32*j+8, :],
                                         h1T_v_prev[:, k, :],
                                         wih2_sb[k][:, 512*j:512*(j+1)],
                                         start=(k == 0), stop=False,
                                         tile_position=(0, 32*j))
                if with_b2:
                    for j in range(4):
                        nc.tensor.matmul(zp2[32*j:32*j+8, :], onesb_row[:1, :8],
                                         b2_sb[:1, 512*j:512*(j+1)],
                                         start=False, stop=False,
                                         tile_position=(0, 32*j))
                for k in range(4):
                    st = (zeros32_v[:, k, :] if w == 2
                          else h2T_v[:, k, 8*(w-3):8*(w-3)+8])
                    for j in range(4):
                        nc.tensor.matmul(zp2[32*j:32*j+8, :], st,
                                         whh2_sb[k][:, 512*j:512*(j+1)],
                                         start=False, stop=(k == 3),
                                         tile_position=(0, 32*j))
                cc2_prev, hh2, _gg2t = lstm_tail(zp2, cc2_prev, "2")
                if "hh" in dbg:
                    for j in range(4):
                        nc.gpsimd.dma_start(dbg["hh"][w-2, 8:16, 128*j:128*(j+1)],
                                            hh2[32*j:32*j+8, :])
                hh2_prev = hh2

            if w == (L // 2) + 3 and L >= 128 and upto != "D0":
                emit_vocab(0, R // 256, "a")   # rows for steps < L/2

        # final h2 slot (h2(L-1), from chain2(L+1)'s hh2)
        TB = tps.tile([128, 128], F32, tag="TB")
        nc.tensor.matmul(TB[:], hh2_prev[:], ident128f[:], is_transpose=True)
        for kk in range(4):
            nc.vector.tensor_copy(h2T_v[:, kk, 8*(L-1):8*(L-1)+8],
                                  TB[:, 32*kk:32*kk+8])

        if upto != "D0":
            emit_vocab(R // 256 if L >= 128 else 0, R // 128, "b")
    if upto == "D":
        ctx.close(); return

    ctx.close()


_CACHE = {}


def _get_runner(with_b1, with_b2, reps=1, upto="E"):
    key = (with_b1, with_b2, reps, upto)
    if key in _CACHE:
        return _CACHE[key]
    import jax
    from jax.sharding import Mesh, PartitionSpec
    from jax.experimental.shard_map import shard_map
    from concourse.bass2jax import (_bass_exec_p, install_neuronx_cc_hook,
                                    partition_id_tensor)
    nc = build(debug_outputs=(), upto=upto, with_b1=with_b1, with_b2=with_b2,
               reps=reps)
    nc.compile()
    install_neuronx_cc_hook()
    partition_name = (nc.partition_id_tensor.name
                      if nc.partition_id_tensor else None)
    in_names, out_names, out_avals, zero_shapes = [], [], [], []
    for alloc in nc.m.functions[0].allocations:
        if not isinstance(alloc, mybir.MemoryLocationSet):
            continue
        name = alloc.memorylocations[0].name
        if alloc.kind == "ExternalInput":
            if name != partition_name:
                in_names.append(name)
        elif alloc.kind == "ExternalOutput":
            shape = tuple(alloc.tensor_shape)
            dtype = mybir.dt.np(alloc.dtype)
            out_names.append(name)
            out_avals.append(jax.core.ShapedArray(shape, dtype))
            zero_shapes.append((shape, dtype))
    n_params, n_outs = len(in_names), len(out_avals)
    all_in_names = in_names + out_names
    if partition_name is not None:
        all_in_names.append(partition_name)
    donate = tuple(range(n_params, n_params + n_outs))

    def _body(*args):
        operands = list(args)
        if partition_name is not None:
            operands.append(partition_id_tensor())
        outs = _bass_exec_p.bind(
            *operands, out_avals=tuple(out_avals), in_names=tuple(all_in_names),
            out_names=tuple(out_names), lowering_input_output_aliases=(),
            sim_require_finite=False, sim_require_nnan=False, nc=nc)
        return tuple(outs)

    devices = jax.devices()[:N_CORES]
    mesh = Mesh(np.asarray(devices), ("core",))
    sharded = jax.jit(
        shard_map(_body, mesh=mesh,
                  in_specs=(PartitionSpec("core"),) * (n_params + n_outs),
                  out_specs=(PartitionSpec("core"),) * n_outs,
                  check_rep=False),
        donate_argnums=donate, keep_unused=True)
    sharding = jax.sharding.NamedSharding(mesh, PartitionSpec("core"))
    state = {"in_names": in_names, "out_names": out_names,
             "zero_shapes": zero_shapes, "sharded": sharded,
             "sharding": sharding, "out_avals": out_avals}
    _CACHE[key] = state
    return state


def run_device(in_maps, with_b1, with_b2):
    """Run the SPMD kernel; returns (per-core result dicts, wall seconds)."""
    import time as _time
    import jax
    st = _get_runner(with_b1, with_b2)
    concat_in = [np.concatenate([np.asarray(m[name]) for m in in_maps], axis=0)
                 for name in st["in_names"]]
    dev_in = [jax.device_put(a, st["sharding"]) for a in concat_in]
    dev_zeros = [jax.device_put(
        np.zeros((N_CORES * s[0], *s[1:]), dt), st["sharding"])
        for (s, dt) in st["zero_shapes"]]
    for z in dev_zeros:
        z.block_until_ready()
    t0 = _time.perf_counter()
    out_arrs = st["sharded"](*dev_in, *dev_zeros)
    for o in out_arrs:
        o.block_until_ready()
    wall = _time.perf_counter() - t0
    results = [
        {name: np.asarray(out_arrs[i]).reshape(
            N_CORES, *st["out_avals"][i].shape)[c]
         for i, name in enumerate(st["out_names"])}
        for c in range(N_CORES)
    ]
    return results, wall


def kernel(**inputs):
    in_maps = host_prep(inputs)
    b1 = np.asarray(inputs["b_ih1"]) + np.asarray(inputs["b_hh1"])
    b2 = np.asarray(inputs["b_ih2"]) + np.asarray(inputs["b_hh2"])
    results, _ = run_device(in_maps, bool(np.any(b1)), bool(np.any(b2)))
    out = np.concatenate([results[c]["out"] for c in range(N_CORES)], axis=0)
    b_out = np.asarray(inputs["b_out"], np.float32)
    if np.any(b_out):
        out = out + b_out[None, None, :]
    return out


# revision 3
# speedup vs baseline: 1.0598x; 1.0598x over previous
"""Decoder kernel for trn2 — optimized phase D (v2).

Sharding: data-parallel over batch N=64 -> 8 sequences per core.
Per core:
  B. attention (energy/exp-mask/context) per sequence  [f32r matmuls]
  C. z1x = [ce|ctx/s] @ W_ih1p.T (+b1) batched, n-major rows -> DRAM (bf16)
  D. LSTM chains, decoupled pipelines:
     - layout: z/gates/c/h tiles are [128, 128]-style with batch rows at
       partition 32j+r (j = feature chunk, r = batch 0..8); full-partition
       ACT/DVE ops (garbage rows processed for free)
     - one PE transpose per chain per window gives feature-major h for the
       next window's stationaries (no padded stationary copies)
     - chain2 lags chain1 by one window; h2 written straight into h2T_all
  E. vocab projection (bf16), (t,n)-major rows, streamed w_outT in
     1000-col slabs
"""
import sys
sys.path.insert(0, '/opt/trn_rl_repo')
import numpy as np
import ml_dtypes
from concourse import bass, bacc, mybir
from concourse.tile import TileContext
from concourse import masks

F32, F32R, BF16 = mybir.dt.float32, mybir.dt.float32r, mybir.dt.bfloat16
AF = mybir.ActivationFunctionType
ALU = mybir.AluOpType

N_CORES = 8
T, K, V, H, L, VOCAB = 512, 512, 512, 512, 256, 10000
NL = 8
R = NL * L       # 2048 rows per core
MASK_NEG = -30.0
VB = 500         # vocab slice (20 slices, loaded in 1000-col pairs)


PERM = np.concatenate([
    np.concatenate([gate * 512 + np.arange(128 * j, 128 * (j + 1))
                    for gate in (0, 1, 3, 2)])       # i, f, o, g
    for j in range(4)])


def _bf(x):
    return np.ascontiguousarray(x).astype(ml_dtypes.bfloat16)


def host_prep(inputs):
    key = np.asarray(inputs["key"], np.float32)
    values = np.asarray(inputs["values"], np.float32)
    text = np.asarray(inputs["text"])
    text_lens = np.asarray(inputs["text_lens"])
    emb = np.asarray(inputs["emb"], np.float32)
    w_ih1 = np.asarray(inputs["w_ih1"], np.float32)
    w_hh1 = np.asarray(inputs["w_hh1"], np.float32)
    w_ih2 = np.asarray(inputs["w_ih2"], np.float32)
    w_hh2 = np.asarray(inputs["w_hh2"], np.float32)
    b1 = (np.asarray(inputs["b_ih1"], np.float32)
          + np.asarray(inputs["b_hh1"], np.float32))
    b2 = (np.asarray(inputs["b_ih2"], np.float32)
          + np.asarray(inputs["b_hh2"], np.float32))
    w_out = np.asarray(inputs["w_out"], np.float32)

    w_ih1p, w_hh1p, b1p = w_ih1[PERM], w_hh1[PERM], b1[PERM]
    w_ih2p, w_hh2p, b2p = w_ih2[PERM], w_hh2[PERM], b2[PERM]

    mask = (np.arange(T)[None, :] < text_lens[:, None])
    maskb = np.where(mask, 0.0, MASK_NEG).astype(np.float32)

    ce_all = emb[text[:, :L]]

    shared = {
        "w1ceT": np.ascontiguousarray(w_ih1p[:, :512].T),
        "w1ctxT": np.ascontiguousarray(w_ih1p[:, 512:].T),
        "b1": b1p.reshape(1, 2048),
        "whh1T": _bf(w_hh1p.T),          # (512, 2048) bf16
        "wih2T": _bf(w_ih2p.T),
        "whh2T": _bf(w_hh2p.T),
        "b2bf": _bf(b2p.reshape(1, 2048)),
        "w_outT": _bf(w_out.T),          # (1024, 10000) bf16
    }
    in_maps = []
    for c in range(N_CORES):
        sl = slice(8 * c, 8 * c + 8)
        ceT = np.ascontiguousarray(ce_all[sl].reshape(R, H).T)
        keyT = np.ascontiguousarray(key[:, sl, :].transpose(1, 2, 0))
        vals = np.ascontiguousarray(values[:, sl, :])
        valT = _bf(values[:L, sl, :].reshape(R, V).T)
        m = {"ceT": ceT, "keyT": keyT, "vals": vals, "valT": valT,
             "maskb": np.ascontiguousarray(maskb[sl]), **shared}
        in_maps.append(m)
    return in_maps


def build(debug_outputs=(), upto="E", with_b1=True, with_b2=False, reps=1):
    nc = bacc.Bacc("TRN2", target_bir_lowering=False, debug=False,
                   num_devices=N_CORES)
    d = {}
    def din(name, shape, dt=F32):
        d[name] = nc.dram_tensor(name, list(shape), dt, kind="ExternalInput")
    din("ceT", (H, R)); din("keyT", (NL, K, T)); din("vals", (T, NL, V))
    din("maskb", (NL, T))  # L-indep
    din("w1ceT", (512, 2048)); din("w1ctxT", (512, 2048))
    din("b1", (1, 2048))
    din("whh1T", (512, 2048), BF16); din("wih2T", (512, 2048), BF16)
    din("whh2T", (512, 2048), BF16); din("b2bf", (1, 2048), BF16)
    din("valT", (V, R), BF16); din("w_outT", (1024, VOCAB), BF16)

    out = nc.dram_tensor("out", [NL, L, VOCAB], F32, kind="ExternalOutput")
    dbg = {}
    shapes = {"att": (NL, T, L), "ctxT": (V, R), "recip": (NL, L),
              "z1x": (L, NL, 2048), "hh": (L, 2 * NL, H),
              "gg1": (L, NL, 2048), "zq0": (NL, 2 * 2048),
              "ht0": (128, 32)}
    for name in debug_outputs:
        dbg[name] = nc.dram_tensor("dbg_" + name, list(shapes[name]), F32,
                                   kind="ExternalOutput")

    with TileContext(nc) as tc:
        for _ in range(reps):
            build_body(nc, tc, d, out, dbg, upto, with_b1, with_b2)
    return nc


def build_body(nc, tc, d, out, dbg, upto, with_b1, with_b2):
    from contextlib import ExitStack
    ctx = ExitStack()
    pool = ctx.enter_context(tc.tile_pool(name="main", bufs=1))
    dramp = ctx.enter_context(tc.tile_pool(name="drp", bufs=1, space="DRAM"))

    # ---- constants ----
    ones_f = pool.tile([128, 128], F32, tag="ones_f")
    nc.gpsimd.memset(ones_f[:], 1.0)
    ones_row = pool.tile([1, 128], F32R, tag="ones_row")
    nc.vector.tensor_copy(ones_row[:], ones_f[0:1, :])
    ones_col = pool.tile([128, 1], F32R, tag="ones_col")
    nc.vector.tensor_copy(ones_col[:], ones_f[:, 0:1])
    ident8f = pool.tile([8, 8], F32, tag="ident8f")
    masks.make_identity(nc, ident8f[:])
    ident8b = pool.tile([8, 8], BF16, tag="ident8b")
    masks.make_identity(nc, ident8b[:])
    ident128f = pool.tile([128, 128], F32, tag="ident128f")
    masks.make_identity(nc, ident128f[:])
    onesb_row = pool.tile([1, 8], BF16, tag="onesb_row")
    nc.gpsimd.memset(onesb_row[:], 1.0)

    z1x_dram = dramp.tile([L, NL, 2048], BF16, tag="z1x_dram")

    # ================= Phase B/C scope =================
    from contextlib import ExitStack as _ES
    with tc.tile_pool(name="bc", bufs=1) as bcp, \
         tc.tile_pool(name="bcps", bufs=4, space="PSUM") as psum, \
         _ES() as bstack:
        attp = bstack.enter_context(tc.tile_pool(name="attp", bufs=2))
        ceT_sb = [bcp.tile([128, R], F32R, tag=f"ceT{kc}", name=f"ceT{kc}")
                  for kc in range(4)]
        for kc in range(4):
            nc.sync.dma_start(ceT_sb[kc][:],
                              d["ceT"][128*kc:128*(kc+1), :].bitcast(F32R))
        ctxT_sb = [bcp.tile([128, R], F32R, tag=f"ctxT{vc}", name=f"ctxT{vc}")
                   for vc in range(4)]
        recipT_sb = bcp.tile([128, 16], F32, tag="recipT")
        sums_sb = bcp.tile([1, NL * L], F32, tag="sums")

        # ---- Phase B: attention ----
        for n in range(NL):
            keyT_n = attp.tile([128, 4 * T], F32R, tag="keyT_n")
            for kc in range(4):
                nc.sync.dma_start(keyT_n[:, T*kc:T*(kc+1)],
                                  d["keyT"][n, 128*kc:128*(kc+1), :].bitcast(F32R))
            maskb_n = attp.tile([128, 4], F32, tag="maskb_n")
            nc.sync.dma_start(maskb_n[:],
                              d["maskb"][n, :].rearrange("(a b) -> b a", b=128))
            att_n = attp.tile([128, 4 * L], F32R, tag="att_n")
            for tch in range(4):
                ep = psum.tile([128, L], F32, tag="mm")
                for kc in range(4):
                    nc.tensor.matmul(ep[:],
                                     keyT_n[:, T*kc+128*tch:T*kc+128*tch+128],
                                     ceT_sb[kc][:, L*n:L*(n+1)],
                                     start=(kc == 0), stop=(kc == 3))
                nc.scalar.activation(att_n[:, L*tch:L*(tch+1)], ep[:], AF.Exp,
                                     bias=maskb_n[:, tch:tch+1])
            sp = psum.tile([1, L], F32, tag="sp", bufs=1)
            for tch in range(4):
                nc.tensor.matmul(sp[:], ones_col[:, :1],
                                 att_n[:, L*tch:L*(tch+1)],
                                 start=(tch == 0), stop=(tch == 3))
            nc.scalar.activation(sums_sb[0:1, L*n:L*(n+1)], sp[:], AF.Copy)
            vals_n = attp.tile([128, 4 * V], F32R, tag="vals_n")
            for tch in range(4):
                nc.gpsimd.dma_start(vals_n[:, V*tch:V*(tch+1)],
                                    d["vals"][128*tch:128*(tch+1), n, :].bitcast(F32R))
            for vc in range(4):
                cp = psum.tile([128, L], F32, tag="mm")
                for tch in range(4):
                    nc.tensor.matmul(cp[:],
                                     vals_n[:, V*tch+128*vc:V*tch+128*vc+128],
                                     att_n[:, L*tch:L*(tch+1)],
                                     start=(tch == 0), stop=(tch == 3))
                nc.scalar.activation(ctxT_sb[vc][:, L*n:L*(n+1)], cp[:], AF.Copy)
            if "att" in dbg:
                af = attp.tile([128, 4 * L], F32, tag="dbgf", bufs=1)
                nc.vector.tensor_copy(af[:], att_n[:].bitcast(F32))
                for tch in range(4):
                    nc.sync.dma_start(dbg["att"][n, 128*tch:128*(tch+1), :],
                                      af[:, L*tch:L*(tch+1)])

        recip_nb = bcp.tile([1, NL * L], F32, tag="recip_nb")
        nc.vector.reciprocal(recip_nb[:], sums_sb[:])
        rcols = min(L, 128)
        nchk = max(L // 128, 1)
        for n in range(NL):
            for hh in range(nchk):
                rp = psum.tile([128, 1], F32, tag="rp", bufs=2)
                nc.tensor.matmul(rp[:rcols, :],
                                 recip_nb[0:1, L*n+rcols*hh:L*n+rcols*(hh+1)],
                                 ident8f[0:1, 0:1], is_transpose=True)
                nc.scalar.activation(recipT_sb[:rcols, 8*hh+n:8*hh+n+1],
                                     rp[:rcols, :], AF.Copy)
        if "recip" in dbg:
            nc.sync.dma_start(dbg["recip"][:, :], recip_nb[:])
        if "ctxT" in dbg:
            for vc in range(4):
                cf = attp.tile([128, R], F32, tag="dbgf", bufs=1)
                nc.vector.tensor_copy(cf[:], ctxT_sb[vc][:].bitcast(F32))
                nc.sync.dma_start(dbg["ctxT"][128*vc:128*(vc+1), :], cf[:])
        if upto == "B":
            ctx.close(); return

        # ---- Phase C: z1x (n-major rows; -> z1x_dram step-major bf16) ----
        bstack.close()   # free attention pools before the z1x weights
        zstack = _ES()
        zwp = zstack.enter_context(tc.tile_pool(name="zwp", bufs=1))
        zxp = zstack.enter_context(tc.tile_pool(name="zxp", bufs=4))
        w1ce_sb = [zwp.tile([128, 2048], F32R, tag=f"w1ce{kc}", name=f"w1ce{kc}")
                   for kc in range(4)]
        w1ctx_sb = [zwp.tile([128, 2048], F32R, tag=f"w1ctx{kc}", name=f"w1ctx{kc}")
                    for kc in range(4)]
        b1_sb = zwp.tile([1, 2048], F32R, tag="b1_sb")
        nc.sync.dma_start(b1_sb[:], d["b1"][:].bitcast(F32R))
        for kc in range(4):
            nc.sync.dma_start(w1ce_sb[kc][:],
                              d["w1ceT"][128*kc:128*(kc+1), :].bitcast(F32R))
            nc.sync.dma_start(w1ctx_sb[kc][:],
                              d["w1ctxT"][128*kc:128*(kc+1), :].bitcast(F32R))
        lchunks = max(L // 128, 1)
        crows = min(L, 128)  # rows per (n, lchunk) piece
        for rc in range(R // crows):
            n_of = rc // lchunks
            lh = rc % lchunks
            rs = slice(crows * rc, crows * (rc + 1))
            recip_col = recipT_sb[:crows, 8*lh + n_of: 8*lh + n_of + 1]
            zx_bf = zxp.tile([128, 2048], BF16, tag="zx_bf", bufs=3)
            for j in range(4):
                fs = slice(512 * j, 512 * (j + 1))
                pce = psum.tile([128, 512], F32, tag="mm")
                first = True
                if with_b1:
                    nc.tensor.matmul(pce[:crows, :], ones_row[:1, :crows], b1_sb[:1, fs],
                                     start=True, stop=False)
                    first = False
                for kc in range(4):
                    nc.tensor.matmul(pce[:crows, :], ceT_sb[kc][:, rs],
                                     w1ce_sb[kc][:, fs],
                                     start=(first and kc == 0), stop=(kc == 3))
                pctx = psum.tile([128, 512], F32, tag="mm")
                for kc in range(4):
                    nc.tensor.matmul(pctx[:crows, :], ctxT_sb[kc][:, rs],
                                     w1ctx_sb[kc][:, fs],
                                     start=(kc == 0), stop=(kc == 3))
                zce_sb = zxp.tile([128, 512], F32, tag="zce_sb")
                nc.scalar.activation(zce_sb[:crows, :], pce[:crows, :], AF.Copy)
                nc.vector.scalar_tensor_tensor(out=zx_bf[:crows, fs], in0=pctx[:crows, :],
                                               scalar=recip_col, in1=zce_sb[:crows, :],
                                               op0=ALU.mult, op1=ALU.add)
            # rows (n_of, l=crows*lh + p) -> z1x_dram[l, n, f]
            nc.sync.dma_start(z1x_dram[crows*lh:crows*(lh+1), n_of, :],
                              zx_bf[:crows, :])
        if "z1x" in dbg:
            for t in range(0, L, 8):
                zrb = zxp.tile([8 * NL, 2048], BF16, tag="zrb", bufs=1)
                nc.sync.dma_start(zrb[:], z1x_dram[t:t+8, :, :])
                zrf = zxp.tile([8 * NL, 2048], F32, tag="zrf", bufs=1)
                nc.vector.tensor_copy(zrf[:], zrb[:])
                nc.sync.dma_start(dbg["z1x"][t:t+8, :, :], zrf[:])
        zstack.close()
    if upto == "C":
        ctx.close(); return

    # ================= Phase D: decoupled LSTM chains =================
    # h2T_all[p, (t, k, n)]: p = feature within chunk, k = feature chunk,
    # t = step, n = batch row. Written directly by chain2's transpose cast.
    h2T_all = pool.tile([128, 4 * R], BF16, tag="h2T_all")
    h2T_v = h2T_all[:].rearrange("p (k r) -> p k r", k=4)

    with tc.tile_pool(name="dph", bufs=1) as dph, \
         tc.tile_pool(name="dst", bufs=2) as dst, \
         tc.tile_pool(name="zqp", bufs=3) as zqp, \
         tc.tile_pool(name="zps", bufs=1, space="PSUM") as zps, \
         tc.tile_pool(name="tps", bufs=1, space="PSUM") as tps, \
         tc.tile_pool(name="wvp", bufs=16) as wvp, \
         tc.tile_pool(name="osp", bufs=6) as osp, \
         tc.tile_pool(name="eps", bufs=4, space="PSUM") as eps:
        whh1_sb = [dph.tile([128, 2048], BF16, tag=f"whh1_{k}", name=f"whh1_{k}")
                   for k in range(4)]
        wih2_sb = [dph.tile([128, 2048], BF16, tag=f"wih2_{k}", name=f"wih2_{k}")
                   for k in range(4)]
        whh2_sb = [dph.tile([128, 2048], BF16, tag=f"whh2_{k}", name=f"whh2_{k}")
                   for k in range(4)]
        for k in range(4):
            nc.sync.dma_start(whh1_sb[k][:], d["whh1T"][128*k:128*(k+1), :])
            nc.sync.dma_start(wih2_sb[k][:], d["wih2T"][128*k:128*(k+1), :])
            nc.sync.dma_start(whh2_sb[k][:], d["whh2T"][128*k:128*(k+1), :])
        b2_sb = dph.tile([1, 2048], BF16, tag="b2_sb")
        nc.sync.dma_start(b2_sb[:], d["b2bf"][:])

        valT_sb = dph.tile([128, 4 * R], BF16, tag="valT_sb")
        valT_v = valT_sb[:].rearrange("p (k r) -> p k r", k=4)
        for k in range(4):
            nc.sync.dma_start(valT_v[:, k, :], d["valT"][128*k:128*(k+1), :])

        def emit_vocab(rc_lo, rc_hi, tagp):
            for vp in range(VOCAB // (2 * VB)):
                wts = [wvp.tile([128, 2 * VB], BF16, tag="wv",
                                name=f"wv{tagp}_{vp}_{k}") for k in range(8)]
                for k in range(8):
                    nc.gpsimd.dma_start(wts[k][:],
                                        d["w_outT"][128*k:128*(k+1),
                                                    2*VB*vp:2*VB*(vp+1)])
                for rc in range(rc_lo, rc_hi):
                    osb = osp.tile([128, 2 * VB], F32, tag="osb")
                    for hv in range(2):
                        pv = eps.tile([128, VB], F32, tag="pv")
                        for k in range(4):
                            nc.tensor.matmul(pv[:],
                                             h2T_v[:, k, 128*rc:128*(rc+1)],
                                             wts[k][:, VB*hv:VB*(hv+1)],
                                             start=(k == 0), stop=False)
                        for k in range(4):
                            nc.tensor.matmul(pv[:],
                                             valT_v[:, k, 128*rc:128*(rc+1)],
                                             wts[4+k][:, VB*hv:VB*(hv+1)],
                                             start=False, stop=(k == 3))
                        nc.vector.tensor_copy(osb[:, VB*hv:VB*(hv+1)],
                                              pv[:])
                    nc.sync.dma_start(
                        out[0:NL, 16*rc:16*(rc+1), 2*VB*vp:2*VB*(vp+1)]
                        .transpose([1, 0, 2]), osb[:])

        zeros32 = dph.tile([128, 32], BF16, tag="zeros32")
        nc.gpsimd.memset(zeros32[:], 0.0)
        zeros32_v = zeros32[:].rearrange("p (k c) -> p k c", k=4)
        hh1_init = dph.tile([128, 128], F32, tag="hh1_init")
        nc.vector.memset(hh1_init[:], 0.0)
        cc1_prev = dph.tile([128, 128], F32, tag="cc1_init")
        nc.vector.memset(cc1_prev[:], 0.0)
        cc2_prev = dph.tile([128, 128], F32, tag="cc2_init")
        nc.vector.memset(cc2_prev[:], 0.0)

        hh1_prev, hh2_prev = hh1_init, None
        h1T_v = None
        z1q = None

        def lstm_tail(zp, cc_prev, tag):
            """gates -> c_new, h (bf16 [128,128]); returns (cc_new, hh)."""
            gg = dst.tile([128, 512], F32, tag=f"gg{tag}")
            nc.scalar.activation(gg[:, 0:384], zp[:, 0:384], AF.Sigmoid)
            nc.scalar.activation(gg[:, 384:512], zp[:, 384:512], AF.Tanh)
            t1 = dst.tile([128, 128], F32, tag=f"t1{tag}")
            nc.vector.tensor_tensor(out=t1[:], in0=gg[:, 128:256],
                                    in1=cc_prev[:], op=ALU.mult)
            t2 = dst.tile([128, 128], F32, tag=f"t2{tag}")
            nc.vector.tensor_tensor(out=t2[:], in0=gg[:, 0:128],
                                    in1=gg[:, 384:512], op=ALU.mult)
            cc_new = dst.tile([128, 128], F32, tag=f"cc{tag}")
            nc.vector.tensor_tensor(out=cc_new[:], in0=t1[:], in1=t2[:],
                                    op=ALU.add)
            th = dst.tile([128, 128], F32, tag=f"th{tag}")
            nc.scalar.activation(th[:], cc_new[:], AF.Tanh)
            hh = dst.tile([128, 128], F32, tag=f"hh{tag}")
            nc.vector.tensor_tensor(out=hh[:], in0=gg[:, 256:384],
                                    in1=th[:], op=ALU.mult)
            return cc_new, hh, gg

        h1T_v_cur, h1T_v_prev = None, None
        for w in range(L + 2):
            c1 = w < L
            c2 = 2 <= w <= L + 1      # chain2(w) computes h2(w-2)
            # --- T_A(w): h1(w-1) -> feature-major bf16 [128,(k,8)] ---
            if w <= L:
                TA = tps.tile([128, 128], F32, tag="TA")
                nc.tensor.matmul(TA[:], hh1_prev[:], ident128f[:],
                                 is_transpose=True)
                h1T = dst.tile([128, 32], BF16, tag="h1T")
                h1T_v_prev, h1T_v_cur = (
                    h1T_v_cur, h1T[:].rearrange("p (k c) -> p k c", k=4))
                h1T_v = h1T_v_cur
                for kk in range(4):
                    nc.vector.tensor_copy(h1T[:, 8*kk:8*(kk+1)],
                                          TA[:, 32*kk:32*kk+8])

            # --- chain1: z1(w) = z1x(w) + h1(w-1) @ whh1 ---
            if c1:
                z1q = zqp.tile([8, 2048], BF16, tag="z1q")
                nc.sync.dma_start(z1q[:], z1x_dram[w, :, :])
                zp1 = zps.tile([128, 512], F32, tag="zp1")
                qof = 0
                for j in range(4):
                    nc.tensor.matmul(zp1[32*j:32*j+8, :], ident8b[:],
                                     z1q[:, qof+512*j:qof+512*(j+1)],
                                     start=True, stop=False,
                                     tile_position=(0, 32*j))
                for k in range(4):
                    for j in range(4):
                        nc.tensor.matmul(zp1[32*j:32*j+8, :],
                                         h1T_v[:, k, :],
                                         whh1_sb[k][:, 512*j:512*(j+1)],
                                         start=False, stop=(k == 3),
                                         tile_position=(0, 32*j))
                if w == 0 and "zq0" in dbg:
                    nc.gpsimd.dma_start(dbg["zq0"][:, 0:2048], z1q[:, :])
                if w == 0 and "ht0" in dbg:
                    nc.gpsimd.dma_start(dbg["ht0"][:, :], h1T[:, :])
                cc1_prev, hh1, gg1t = lstm_tail(zp1, cc1_prev, "1")
                if "gg1" in dbg:
                    for j in range(4):
                        nc.sync.dma_start(
                            dbg["gg1"][w, 0:8, 512*j:512*(j+1)],
                            gg1t[32*j:32*j+8, :])
                if "hh" in dbg:
                    for j in range(4):
                        nc.gpsimd.dma_start(dbg["hh"][w, 0:8, 128*j:128*(j+1)],
                                            hh1[32*j:32*j+8, :])
                hh1_prev = hh1

            # --- T_B(w): hh2(w-1) holds h2(w-3) -> h2T slot w-3 (w >= 3) ---
            if w >= 3:
                TB = tps.tile([128, 128], F32, tag="TB")
                nc.tensor.matmul(TB[:], hh2_prev[:], ident128f[:],
                                 is_transpose=True)
                for kk in range(4):
                    nc.vector.tensor_copy(
                        h2T_v[:, kk, 8*(w-3):8*(w-3)+8],
                        TB[:, 32*kk:32*kk+8])

            # --- chain2(w): z2(w-2) = h1(w-2) @ wih2 + h2(w-3) @ whh2 ---
            if c2:
                zp2 = zps.tile([128, 512], F32, tag="zp2")
                for k in range(4):
                    for j in range(4):
                        nc.tensor.matmul(zp2[32*j:32*j+8, :],
                                         h1T_v_prev[:, k, :],
                                         wih2_sb[k][:, 512*j:512*(j+1)],
                                         start=(k == 0), stop=False,
                                         tile_position=(0, 32*j))
                if with_b2:
                    for j in range(4):
                        nc.tensor.matmul(zp2[32*j:32*j+8, :], onesb_row[:1, :8],
                                         b2_sb[:1, 512*j:512*(j+1)],
                                         start=False, stop=False,
                                         tile_position=(0, 32*j))
                for k in range(4):
                    st = (zeros32_v[:, k, :] if w == 2
                          else h2T_v[:, k, 8*(w-3):8*(w-3)+8])
                    for j in range(4):
                        nc.tensor.matmul(zp2[32*j:32*j+8, :], st,
                                         whh2_sb[k][:, 512*j:512*(j+1)],
                                         start=False, stop=(k == 3),
                                         tile_position=(0, 32*j))
                cc2_prev, hh2, _gg2t = lstm_tail(zp2, cc2_prev, "2")
                if "hh" in dbg:
                    for j in range(4):
                        nc.gpsimd.dma_start(dbg["hh"][w-2, 8:16, 128*j:128*(j+1)],
                                            hh2[32*j:32*j+8, :])
                hh2_prev = hh2

            if w == (L // 2) + 3 and L >= 128 and upto != "D0":
                emit_vocab(0, R // 256, "a")   # rows for steps < L/2

        # final h2 slot (h2(L-1), from chain2(L+1)'s hh2)
        TB = tps.tile([128, 128], F32, tag="TB")
        nc.tensor.matmul(TB[:], hh2_prev[:], ident128f[:], is_transpose=True)
        for kk in range(4):
            nc.vector.tensor_copy(h2T_v[:, kk, 8*(L-1):8*(L-1)+8],
                                  TB[:, 32*kk:32*kk+8])

        if upto != "D0":
            emit_vocab(R // 256 if L >= 128 else 0, R // 128, "b")
    if upto == "D":
        ctx.close(); return

    ctx.close()


_CACHE = {}


def _get_runner(with_b1, with_b2, reps=1, upto="E"):
    key = (with_b1, with_b2, reps, upto)
    if key in _CACHE:
        return _CACHE[key]
    import jax
    from jax.sharding import Mesh, PartitionSpec
    from jax.experimental.shard_map import shard_map
    from concourse.bass2jax import (_bass_exec_p, install_neuronx_cc_hook,
                                    partition_id_tensor)
    nc = build(debug_outputs=(), upto=upto, with_b1=with_b1, with_b2=with_b2,
               reps=reps)
    nc.compile()
    install_neuronx_cc_hook()
    partition_name = (nc.partition_id_tensor.name
                      if nc.partition_id_tensor else None)
    in_names, out_names, out_avals, zero_shapes = [], [], [], []
    for alloc in nc.m.functions[0].allocations:
        if not isinstance(alloc, mybir.MemoryLocationSet):
            continue
        name = alloc.memorylocations[0].name
        if alloc.kind == "ExternalInput":
            if name != partition_name:
                in_names.append(name)
        elif alloc.kind == "ExternalOutput":
            shape = tuple(alloc.tensor_shape)
            dtype = mybir.dt.np(alloc.dtype)
            out_names.append(name)
            out_avals.append(jax.core.ShapedArray(shape, dtype))
            zero_shapes.append((shape, dtype))
    n_params, n_outs = len(in_names), len(out_avals)
    all_in_names = in_names + out_names
    if partition_name is not None:
        all_in_names.append(partition_name)
    donate = tuple(range(n_params, n_params + n_outs))

    def _body(*args):
        operands = list(args)
        if partition_name is not None:
            operands.append(partition_id_tensor())
        outs = _bass_exec_p.bind(
            *operands, out_avals=tuple(out_avals), in_names=tuple(all_in_names),
            out_names=tuple(out_names), lowering_input_output_aliases=(),
            sim_require_finite=False, sim_require_nnan=False, nc=nc)
        return tuple(outs)

    devices = jax.devices()[:N_CORES]
    mesh = Mesh(np.asarray(devices), ("core",))
    sharded = jax.jit(
        shard_map(_body, mesh=mesh,
                  in_specs=(PartitionSpec("core"),) * (n_params + n_outs),
                  out_specs=(PartitionSpec("core"),) * n_outs,
                  check_rep=False),
        donate_argnums=donate, keep_unused=True)
    sharding = jax.sharding.NamedSharding(mesh, PartitionSpec("core"))
    state = {"in_names": in_names, "out_names": out_names,
             "zero_shapes": zero_shapes, "sharded": sharded,
             "sharding": sharding, "out_avals": out_avals}
    _CACHE[key] = state
    return state


def run_device(in_maps, with_b1, with_b2):
    """Run the SPMD kernel; returns (per-core result dicts, wall seconds)."""
    import time as _time
    import jax
    st = _get_runner(with_b1, with_b2)
    concat_in = [np.concatenate([np.asarray(m[name]) for m in in_maps], axis=0)
                 for name in st["in_names"]]
    dev_in = [jax.device_put(a, st["sharding"]) for a in concat_in]
    dev_zeros = [jax.device_put(
        np.zeros((N_CORES * s[0], *s[1:]), dt), st["sharding"])
        for (s, dt) in st["zero_shapes"]]
    for z in dev_zeros:
        z.block_until_ready()
    t0 = _time.perf_counter()
    out_arrs = st["sharded"](*dev_in, *dev_zeros)
    for o in out_arrs:
        o.block_until_ready()
    wall = _time.perf_counter() - t0
    results = [
        {name: np.asarray(out_arrs[i]).reshape(
            N_CORES, *st["out_avals"][i].shape)[c]
         for i, name in enumerate(st["out_names"])}
        for c in range(N_CORES)
    ]
    return results, wall


def kernel(**inputs):
    in_maps = host_prep(inputs)
    b1 = np.asarray(inputs["b_ih1"]) + np.asarray(inputs["b_hh1"])
    b2 = np.asarray(inputs["b_ih2"]) + np.asarray(inputs["b_hh2"])
    results, _ = run_device(in_maps, bool(np.any(b1)), bool(np.any(b2)))
    out = np.concatenate([results[c]["out"] for c in range(N_CORES)], axis=0)
    b_out = np.asarray(inputs["b_out"], np.float32)
    if np.any(b_out):
        out = out + b_out[None, None, :]
    return out
